# revision 9
# baseline (speedup 1.0000x reference)
"""Trainium2 Bass kernel for nn_Deformation (deep-snake deformation network).

Strategy (pure data-parallel over batch, 2 batches per core x 8 cores):
  * Host: build zero-padded channels-last 2x2-patch "site tables" per feature
    level so grid_sample becomes ONE row-gather per vertex per level
    (4 pixels x C channels contiguous, bf16).
  * Device, per refinement step:
      - compute gather indices + bilinear weights from vertices (fp32, exact
        floor via int-cast + correction),
      - SWDGE dma_gather rows into verts-on-partition tiles,
      - bilinear-combine with per-partition scalar_tensor_tensor chains,
      - PE-transpose to channel-major pf, 1x1 fuse conv,
      - dilated circular conv stack as 9 shifted matmuls/block with PSUM
        accumulation; BN folded into per-channel bias/scale (exact affine
        refactoring, relu on ACT, residual+scale via scalar_tensor_tensor),
      - fusion conv + free-dim max-reduce; global feature folded into pred0
        bias via a tiny matmul,
      - pred0/1/2 1x1 convs, vertex update in fp32.
  * Weights/activations bf16 (fp32 PSUM accumulate); vertices fp32 end-to-end.
"""

import numpy as np
import ml_dtypes

import concourse.bacc as bacc
import concourse.mybir as mybir
from concourse import bass_utils
from concourse.tile import TileContext

bf16 = mybir.dt.bfloat16
f32 = mybir.dt.float32
i16 = mybir.dt.int16
i32 = mybir.dt.int32
AL = mybir.AluOpType
AF = mybir.ActivationFunctionType
AX = mybir.AxisListType

B, N = 16, 1024
NCORES = 8
BPC = B // NCORES            # batches per core
NV = BPC * N                 # vertex columns per core
LEVELS = [(64, 128), (128, 64), (256, 32), (512, 16)]  # (C, W=H) per level
CB = [0, 64, 192, 448]       # channel base of each level inside pf (960 ch)
SL = [(w + 2) * (w + 2) for _, w in LEVELS]  # site-table rows per batch
DILS = [1, 1, 1, 2, 2, 4, 4]
BN_EPS = 1e-5

# gather groups: (level, jj0, num_idx). jj = global 128-vertex tile (0..15).
GROUPS = (
    [(0, 0, 1024), (0, 8, 1024)]
    + [(1, jj0, 512) for jj0 in (0, 4, 8, 12)]
    + [(2, jj0, 512) for jj0 in (0, 4, 8, 12)]
    + [(3, jj0, 256) for jj0 in range(0, 16, 2)]
)


# --------------------------------------------------------------------------
# device program
# --------------------------------------------------------------------------

def build_program(nc):
    S0 = SL[0]
    # ---- DRAM tensors ----
    taba = [
        nc.dram_tensor("tab0a", [S0, 256], bf16, kind="ExternalInput"),
        nc.dram_tensor("tab0b", [S0, 256], bf16, kind="ExternalInput"),
    ]
    tabs = [
        None,
        nc.dram_tensor("tab1", [2 * SL[1], 512], bf16, kind="ExternalInput"),
        nc.dram_tensor("tab2", [2 * SL[2], 1024], bf16, kind="ExternalInput"),
        nc.dram_tensor("tab3", [2 * SL[3], 2048], bf16, kind="ExternalInput"),
    ]
    verts_d = nc.dram_tensor("vertsrow", [2, NV], f32, kind="ExternalInput")
    id2_d = nc.dram_tensor("id2", [2, 2], f32, kind="ExternalInput")
    idn_d = nc.dram_tensor("idn", [128, 128], bf16, kind="ExternalInput")
    sc_d = nc.dram_tensor("SC", [128, 4, 32], f32, kind="ExternalInput")
    boff_d = nc.dram_tensor("BOFF", [128, 4, 16], f32, kind="ExternalInput")
    fuse7_d = nc.dram_tensor("fuse7", [128, 7, 64], bf16, kind="ExternalInput")
    fuset_d = nc.dram_tensor("fuset", [64, 64], bf16, kind="ExternalInput")
    bfuse_d = nc.dram_tensor("bfuse", [64, 1], f32, kind="ExternalInput")
    wd, bd = [], []
    for s in range(3):
        wd.append({
            "h": nc.dram_tensor(f"h_lhsT{s}", [66, 9, 128], bf16, kind="ExternalInput"),
            "r": nc.dram_tensor(f"r_lhsT{s}", [128, 7, 9, 128], bf16, kind="ExternalInput"),
            "fu": nc.dram_tensor(f"fu_lhsT{s}", [128, 8, 2, 128], bf16, kind="ExternalInput"),
            "p0": nc.dram_tensor(f"p0_lhsT{s}", [128, 8, 2, 128], bf16, kind="ExternalInput"),
            "g": nc.dram_tensor(f"g_lhsT{s}", [128, 2, 2, 128], f32, kind="ExternalInput"),
            "p1": nc.dram_tensor(f"p1_lhsT{s}", [128, 2, 64], bf16, kind="ExternalInput"),
            "p2": nc.dram_tensor(f"p2_lhsT{s}", [64, 2], bf16, kind="ExternalInput"),
        })
        bd.append({
            "bact": nc.dram_tensor(f"bact{s}", [128, 8], f32, kind="ExternalInput"),
            "binv": nc.dram_tensor(f"binv{s}", [128, 8], f32, kind="ExternalInput"),
            "bfu": nc.dram_tensor(f"bfu{s}", [128, 2], f32, kind="ExternalInput"),
            "b0": nc.dram_tensor(f"b0{s}", [128, 2], f32, kind="ExternalInput"),
            "b1": nc.dram_tensor(f"b1{s}", [64, 1], f32, kind="ExternalInput"),
            "b2": nc.dram_tensor(f"b2{s}", [2, 1], f32, kind="ExternalInput"),
        })
    pred_d = [nc.dram_tensor(f"pred{s}", [2, NV], f32, kind="ExternalOutput") for s in range(3)]
    bounce_d = [nc.dram_tensor(f"bounce{s}", [128 * 64], i16, kind="Internal") for s in range(3)]

    with TileContext(nc) as tc:
        with (
            tc.tile_pool(name="const", bufs=1) as cp,
            tc.tile_pool(name="wts", bufs=1) as wp,
            tc.tile_pool(name="small", bufs=2) as sp,
            tc.tile_pool(name="g01", bufs=2) as gp01,
            tc.tile_pool(name="g2", bufs=2) as gp2,
            tc.tile_pool(name="g3", bufs=2) as gp3,
            tc.tile_pool(name="pfv", bufs=1) as pfvp,
            tc.tile_pool(name="pfc", bufs=1) as pfcp,
            tc.tile_pool(name="xe", bufs=1) as xep,
            tc.tile_pool(name="rr", bufs=2) as rp,
            tc.tile_pool(name="h01", bufs=1) as hp,
            tc.tile_pool(name="ps_acc", bufs=2, space="PSUM") as psa,
            tc.tile_pool(name="ps_mini", bufs=2, space="PSUM") as pss,
        ):
            # ---- constants ----
            id2 = cp.tile([2, 2], f32)
            nc.sync.dma_start(id2[:, :], id2_d.ap())
            idn = cp.tile([128, 128], bf16)
            nc.sync.dma_start(idn[:, :], idn_d.ap())
            sct = cp.tile([128, 4, 32], f32)
            nc.sync.dma_start(sct[:, :, :], sc_d.ap())
            bofft = cp.tile([128, 4, 16], f32)
            nc.sync.dma_start(bofft[:, :, :], boff_d.ap())
            fuse7 = cp.tile([128, 7, 64], bf16)
            nc.sync.dma_start(fuse7[:, :, :], fuse7_d.ap())
            fuset = cp.tile([64, 64], bf16)
            nc.sync.dma_start(fuset[:, :], fuset_d.ap())
            bfuse = cp.tile([64, 1], f32)
            nc.sync.dma_start(bfuse[:, :], bfuse_d.ap())
            bt = []
            for s in range(3):
                d = {}
                for k, shp in [("bact", [128, 8]), ("binv", [128, 8]), ("bfu", [128, 2]),
                               ("b0", [128, 2]), ("b1", [64, 1]), ("b2", [2, 1])]:
                    d[k] = cp.tile(shp, f32, tag=f"{k}{s}", name=f"{k}{s}")
                    nc.sync.dma_start(d[k][(slice(None), slice(None))], bd[s][k].ap())
                bt.append(d)

            # initial vertices (single tile, updated in place each step)
            vr = sp.tile([2, NV], f32, tag="vr", bufs=1)
            nc.sync.dma_start(vr[:, :], verts_d.ap())

            for s in range(3):
                # ---- snake weights (double-buffered across steps) ----
                wh = wp.tile([66, 9, 128], bf16, tag="wh")
                nc.sync.dma_start(wh[:, :, :], wd[s]["h"].ap())
                wr = wp.tile([128, 7, 9, 128], bf16, tag="wr")
                nc.sync.dma_start(wr[:, :, :, :], wd[s]["r"].ap())
                wfu = wp.tile([128, 8, 2, 128], bf16, tag="wfu")
                nc.sync.dma_start(wfu[:, :, :, :], wd[s]["fu"].ap())
                wp0 = wp.tile([128, 8, 2, 128], bf16, tag="wp0")
                nc.sync.dma_start(wp0[:, :, :, :], wd[s]["p0"].ap())
                wg = wp.tile([128, 2, 2, 128], f32, tag="wg")
                nc.sync.dma_start(wg[:, :, :, :], wd[s]["g"].ap())
                wp1 = wp.tile([128, 2, 64], bf16, tag="wp1")
                nc.sync.dma_start(wp1[:, :, :], wd[s]["p1"].ap())
                wp2 = wp.tile([64, 2], bf16, tag="wp2")
                nc.sync.dma_start(wp2[:, :], wd[s]["p2"].ap())

                # ---- vertices -> (128, 16, 2) compact layout via PE transpose ----
                vxy = sp.tile([128, 16, 2], f32, tag="vxy")
                for grp in range(4):
                    pst = pss.tile([128, 512], f32, tag="mini", name="ps_vxy")
                    for j4 in range(4):
                        jj = grp * 4 + j4
                        nc.tensor.transpose(
                            pst[:, j4 * 2:(j4 + 1) * 2], vr[:, jj * 128:(jj + 1) * 128], id2[:, :]
                        )
                    nc.scalar.copy(vxy[:, grp * 4:(grp + 1) * 4, :], pst[:, 0:8].rearrange("p (a b) -> p a b", b=2))

                # ---- per-level gather index + bilinear weight math ----
                cidx = sp.tile([128, 64], i16, tag="cidx")
                w4s = []
                vxyf = vxy[:, :, :].rearrange("p a b -> p (a b)")
                for lev in range(4):
                    C, W = LEVELS[lev]
                    W2 = W + 2
                    g1 = sp.tile([128, 32], f32, tag="g1")
                    nc.vector.tensor_tensor(g1[:, :], vxyf, sct[:, lev, :], AL.mult)
                    nc.vector.tensor_scalar(g1[:, :], g1[:, :], 0.5, float(W + 1), AL.add, AL.min)
                    nc.vector.tensor_scalar(g1[:, :], g1[:, :], 0.0, None, AL.max)
                    pxi = sp.tile([128, 32], i32, tag="pxi")
                    nc.vector.tensor_copy(pxi[:, :], g1[:, :])
                    pxf = sp.tile([128, 32], f32, tag="pxf")
                    nc.vector.tensor_copy(pxf[:, :], pxi[:, :])
                    dd = sp.tile([128, 32], f32, tag="dd")
                    nc.vector.tensor_tensor(dd[:, :], g1[:, :], pxf[:, :], AL.subtract)
                    mm_ = sp.tile([128, 32], f32, tag="mm_")
                    nc.vector.tensor_scalar(mm_[:, :], dd[:, :], 0.0, None, AL.is_lt)
                    f1 = sp.tile([128, 32], f32, tag="f1")
                    nc.vector.tensor_tensor(f1[:, :], dd[:, :], mm_[:, :], AL.add)
                    px = sp.tile([128, 32], f32, tag="px")
                    nc.vector.tensor_tensor(px[:, :], pxf[:, :], mm_[:, :], AL.subtract)
                    f0 = sp.tile([128, 32], f32, tag="f0")
                    nc.vector.tensor_scalar(f0[:, :], f1[:, :], -1.0, 1.0, AL.mult, AL.add)
                    px3 = px[:, :].rearrange("p (a b) -> p a b", b=2)
                    f13 = f1[:, :].rearrange("p (a b) -> p a b", b=2)
                    f03 = f0[:, :].rearrange("p (a b) -> p a b", b=2)
                    idxf = sp.tile([128, 16], f32, tag="idxf")
                    nc.vector.scalar_tensor_tensor(
                        idxf[:, :], px3[:, :, 1], float(W2), px3[:, :, 0], AL.mult, AL.add
                    )
                    if lev > 0:
                        nc.vector.tensor_tensor(idxf[:, :], idxf[:, :], bofft[:, lev, :], AL.add)
                    nc.vector.tensor_copy(cidx[:, lev * 16:(lev + 1) * 16], idxf[:, :])
                    w4 = sp.tile([128, 16, 4], f32, tag=f"w4_{lev}")
                    nc.vector.tensor_tensor(w4[:, :, 0], f03[:, :, 0], f03[:, :, 1], AL.mult)
                    nc.vector.tensor_tensor(w4[:, :, 1], f13[:, :, 0], f03[:, :, 1], AL.mult)
                    nc.vector.tensor_tensor(w4[:, :, 2], f03[:, :, 0], f13[:, :, 1], AL.mult)
                    nc.vector.tensor_tensor(w4[:, :, 3], f13[:, :, 0], f13[:, :, 1], AL.mult)
                    w4s.append(w4)

                # ---- index relayout: compact -> 16-wrapped + replicated ----
                nc.sync.dma_start(bounce_d[s].ap().rearrange("(p c) -> p c", p=128), cidx[:, :])
                wi = sp.tile([128, 512], i16, tag="wi")
                nc.sync.dma_start(
                    wi[0:16, :].rearrange("q (lev jj g) -> q lev jj g", lev=4, jj=16, g=8),
                    bounce_d[s].ap().rearrange("(g q lev jj) -> q lev jj g", g=8, q=16, lev=4, jj=16),
                )
                nc.sync.dma_start(wi[16:32, :], wi[0:16, :])
                nc.sync.dma_start(wi[32:64, :], wi[0:32, :])
                nc.sync.dma_start(wi[64:128, :], wi[0:64, :])

                # ---- gathers + bilinear combine ----
                pfv = pfvp.tile([128, 16, 960], bf16, tag="pfv")
                for (lev, jj0, nidx) in GROUPS:
                    C, W = LEVELS[lev]
                    ns = nidx // 128
                    pool = gp01 if lev <= 1 else (gp2 if lev == 2 else gp3)
                    gt = pool.tile([128, ns, 4 * C], bf16, tag=f"gt{lev}")
                    if lev == 0:
                        in_ap = taba[jj0 // 8].ap()
                    else:
                        in_ap = tabs[lev].ap()
                    nc.gpsimd.dma_gather(
                        gt[:, :, :], in_ap,
                        wi[:, lev * 128 + jj0 * 8: lev * 128 + jj0 * 8 + nidx // 16],
                        nidx, nidx, 4 * C,
                    )
                    w4 = w4s[lev]
                    for sl in range(ns):
                        jj = jj0 + sl
                        dst = pfv[:, jj, CB[lev]:CB[lev] + C]
                        nc.vector.tensor_scalar(
                            dst, gt[:, sl, 3 * C:4 * C], w4[:, jj, 3:4], None, AL.mult
                        )
                        for k in (2, 1, 0):
                            nc.vector.scalar_tensor_tensor(
                                dst, gt[:, sl, k * C:(k + 1) * C], w4[:, jj, k:k + 1], dst,
                                AL.mult, AL.add,
                            )

                # ---- transpose pf_v -> channel-major chunks, fused with fuse conv ----
                psf = [psa.tile([128, 1024], f32, tag="acc", name=f"psf{b_}") for b_ in range(2)]
                for k in range(8):
                    cw = 128 if k < 7 else 64
                    pfck = pfcp.tile([cw, 2048], bf16, tag="pfck" if k < 7 else "pfct", bufs=3 if k < 7 else 1)
                    for grp in range(4):
                        pst = pss.tile([128, 512], bf16, tag="mini", name="ps_tp")
                        for j4 in range(4):
                            jj = grp * 4 + j4
                            nc.tensor.transpose(
                                pst[0:cw, j4 * 128:(j4 + 1) * 128],
                                pfv[:, jj, k * 128:k * 128 + cw], idn[:, :],
                            )
                        nc.scalar.copy(pfck[:, grp * 512:(grp + 1) * 512], pst[0:cw, :])
                    lw = fuse7[:, k, :] if k < 7 else fuset[:, :]
                    for b_ in range(2):
                        for sub in range(2):
                            nc.tensor.matmul(
                                psf[b_][0:64, sub * 512:(sub + 1) * 512], lw,
                                pfck[:, b_ * 1024 + sub * 512: b_ * 1024 + (sub + 1) * 512],
                                start=(k == 0), stop=(k == 7),
                            )

                # ---- feat assembly (fused 64ch + loc 2ch), circular halo layout ----
                feat = xep.tile([66, 2112], bf16, tag="xe0")
                for b_ in range(2):
                    mid = slice(16 + b_ * 1056, 16 + b_ * 1056 + 1024)
                    nc.vector.tensor_scalar(
                        feat[0:64, mid], psf[b_][0:64, :], bfuse[:, 0:1], None, AL.add
                    )
                    mn = sp.tile([2, 1], f32, tag="vmin")
                    nc.vector.tensor_reduce(mn[:, :], vr[:, b_ * 1024:(b_ + 1) * 1024], AX.X, AL.min)
                    nc.vector.tensor_scalar(
                        feat[64:66, mid], vr[:, b_ * 1024:(b_ + 1) * 1024], mn[:, 0:1], 0.25,
                        AL.subtract, AL.mult,
                    )
                for b_ in range(2):
                    o = b_ * 1056
                    nc.vector.tensor_copy(feat[:, o:o + 16], feat[:, o + 1024:o + 1040])
                    nc.vector.tensor_copy(feat[:, o + 1040:o + 1056], feat[:, o + 16:o + 32])

                # ---- conv stack: head + 7 residual blocks ----
                prev = feat
                xes = []
                for blk in range(8):
                    dil = 1 if blk == 0 else DILS[blk - 1]
                    xe = xep.tile([128, 2112], bf16, tag=f"xe{blk + 1}")
                    for b_ in range(2):
                        psc = psa.tile([128, 1024], f32, tag="acc", name="ps_conv")
                        for t in range(9):
                            lw = wh[:, t, :] if blk == 0 else wr[:, blk - 1, t, :]
                            base = 16 + b_ * 1056 + (t - 4) * dil
                            for sub in range(2):
                                nc.tensor.matmul(
                                    psc[:, sub * 512:(sub + 1) * 512],
                                    lw, prev[:, base + sub * 512: base + (sub + 1) * 512],
                                    start=(t == 0), stop=(t == 8),
                                )
                        r = rp.tile([128, 1024], bf16, tag="r")
                        nc.scalar.activation(r[:, :], psc[:, :], AF.Relu,
                                             bias=bt[s]["bact"][:, blk:blk + 1], scale=1.0)
                        xm = xe[:, 16 + b_ * 1056: 16 + b_ * 1056 + 1024]
                        if blk == 0:
                            nc.vector.tensor_scalar(xm, r[:, :], bt[s]["binv"][:, 0:1], None, AL.mult)
                        else:
                            pm = prev[:, 16 + b_ * 1056: 16 + b_ * 1056 + 1024]
                            nc.vector.scalar_tensor_tensor(
                                xm, r[:, :], bt[s]["binv"][:, blk:blk + 1], pm, AL.mult, AL.add
                            )
                        o = b_ * 1056
                        nc.vector.tensor_copy(xe[:, o:o + 16], xe[:, o + 1024:o + 1040])
                        nc.vector.tensor_copy(xe[:, o + 1040:o + 1056], xe[:, o + 16:o + 32])
                    xes.append(xe)
                    prev = xe

                def state_rhs(k, nsub):
                    b_, sub = nsub // 2, nsub % 2
                    base = 16 + b_ * 1056 + sub * 512
                    return xes[k][:, base:base + 512]

                # ---- fusion conv (1024 -> 256) + per-batch max over vertices ----
                gparts = sp.tile([128, 2, 4], f32, tag="gparts")
                for m in range(2):
                    for b_ in range(2):
                        psb = psa.tile([128, 1024], f32, tag="acc", name="ps_fusion")
                        for k in range(8):
                            for sub in range(2):
                                nc.tensor.matmul(
                                    psb[:, sub * 512:(sub + 1) * 512], wfu[:, k, m, :],
                                    state_rhs(k, 2 * b_ + sub), start=(k == 0), stop=(k == 7),
                                )
                        for sub in range(2):
                            nc.vector.tensor_reduce(
                                gparts[:, m, 2 * b_ + sub:2 * b_ + sub + 1],
                                psb[:, sub * 512:(sub + 1) * 512], AX.X, AL.max,
                            )
                gtl = sp.tile([128, 2, 2], f32, tag="gtile")
                for m in range(2):
                    for b_ in range(2):
                        nc.vector.tensor_tensor(
                            gtl[:, m, b_:b_ + 1], gparts[:, m, 2 * b_:2 * b_ + 1],
                            gparts[:, m, 2 * b_ + 1:2 * b_ + 2], AL.max,
                        )
                    nc.vector.tensor_scalar(
                        gtl[:, m, :], gtl[:, m, :], bt[s]["bfu"][:, m:m + 1], None, AL.add
                    )
                # pred0 per-(m,batch) bias = b0' + W0g @ g
                bias0 = sp.tile([128, 2, 2], f32, tag="bias0")
                for mo in range(2):
                    psg = pss.tile([128, 512], f32, tag="mini", name="ps_g")
                    for k2 in range(2):
                        nc.tensor.matmul(
                            psg[:, 0:2], wg[:, k2, mo, :], gtl[:, k2, :],
                            start=(k2 == 0), stop=(k2 == 1),
                        )
                    nc.vector.tensor_scalar(
                        bias0[:, mo, :], psg[:, 0:2], bt[s]["b0"][:, mo:mo + 1], None, AL.add
                    )

                # ---- pred0 (state 1024 -> 256), relu ----
                h0 = []
                for m in range(2):
                    h = hp.tile([128, 2048], bf16, tag=f"h0_{m}", name=f"h0_{m}")
                    for b_ in range(2):
                        psb = psa.tile([128, 1024], f32, tag="acc", name="ps_pred0")
                        for k in range(8):
                            for sub in range(2):
                                nc.tensor.matmul(
                                    psb[:, sub * 512:(sub + 1) * 512], wp0[:, k, m, :],
                                    state_rhs(k, 2 * b_ + sub), start=(k == 0), stop=(k == 7),
                                )
                        nc.scalar.activation(
                            h[:, b_ * 1024:(b_ + 1) * 1024], psb[:, :],
                            AF.Relu, bias=bias0[:, m, b_:b_ + 1], scale=1.0,
                        )
                    h0.append(h)

                # ---- pred1 (256 -> 64), relu ----
                h1 = hp.tile([64, 2048], bf16, tag="h1")
                for b_ in range(2):
                    psp1 = psa.tile([128, 1024], f32, tag="acc", name="ps_p1")
                    for k2 in range(2):
                        for sub in range(2):
                            nc.tensor.matmul(
                                psp1[0:64, sub * 512:(sub + 1) * 512], wp1[:, k2, :],
                                h0[k2][:, b_ * 1024 + sub * 512: b_ * 1024 + (sub + 1) * 512],
                                start=(k2 == 0), stop=(k2 == 1),
                            )
                    nc.scalar.activation(h1[:, b_ * 1024:(b_ + 1) * 1024], psp1[0:64, :],
                                         AF.Relu, bias=bt[s]["b1"][:, 0:1], scale=1.0)

                # ---- pred2 (64 -> 2) + vertex update ----
                for b_ in range(2):
                    psp2 = psa.tile([128, 1024], f32, tag="acc", name="ps_p2")
                    for sub in range(2):
                        nc.tensor.matmul(
                            psp2[0:2, sub * 512:(sub + 1) * 512], wp2[:, :],
                            h1[:, b_ * 1024 + sub * 512: b_ * 1024 + (sub + 1) * 512],
                            start=True, stop=True,
                        )
                    nc.vector.scalar_tensor_tensor(
                        vr[:, b_ * 1024:(b_ + 1) * 1024], psp2[0:2, :], bt[s]["b2"][:, 0:1],
                        vr[:, b_ * 1024:(b_ + 1) * 1024], AL.add, AL.add
                    )
                nc.sync.dma_start(pred_d[s].ap(), vr[:, :])
    return nc


# --------------------------------------------------------------------------
# host-side input preparation
# --------------------------------------------------------------------------

def _site_tables(feats):
    """feats: list of 4 arrays (BPC, C, H, W) fp32 -> per-level patch tables."""
    out = []
    for f in feats:
        _, C, H, W = f.shape
        fe = np.zeros((BPC, H + 3, W + 3, C), np.float32)
        fe[:, 1:H + 1, 1:W + 1, :] = f.transpose(0, 2, 3, 1)
        t = np.concatenate(
            [fe[:, :-1, :-1], fe[:, :-1, 1:], fe[:, 1:, :-1], fe[:, 1:, 1:]], axis=-1
        )  # (BPC, H+2, W+2, 4C)
        out.append(np.ascontiguousarray(t.reshape(BPC, (H + 2) * (W + 2), 4 * C)).astype(ml_dtypes.bfloat16))
    return out


def _prep_shared(params):
    """Weights/biases shared across cores (host fp32 folding, bf16 lhsT)."""
    bf = ml_dtypes.bfloat16
    m = {}
    Wf = np.asarray(params["fuse"]["w"], np.float32)[:, :, 0]   # (64, 960)
    WfT = Wf.T                                                  # (960, 64)
    m["fuse7"] = np.ascontiguousarray(WfT[:896].reshape(7, 128, 64).transpose(1, 0, 2)).astype(bf)
    m["fuset"] = np.ascontiguousarray(WfT[896:]).astype(bf)
    m["bfuse"] = np.asarray(params["fuse"]["b"], np.float32).reshape(64, 1)
    m["id2"] = np.eye(2, dtype=np.float32)
    m["idn"] = np.eye(128, dtype=np.float32).astype(bf)
    for s, sp_ in enumerate(params["snakes"]):
        head, res = sp_["head"], sp_["res"]
        blocks = [head] + list(res)
        inv = [np.asarray(p["g"], np.float32) / np.sqrt(np.asarray(p["v"], np.float32) + BN_EPS) for p in blocks]
        cc = [np.asarray(p["be"], np.float32) - np.asarray(p["m"], np.float32) * iv for p, iv in zip(blocks, inv)]
        Cs = np.cumsum(np.stack(cc), axis=0)   # (8, 128): const after state i
        bact = [np.asarray(head["b"], np.float32)]
        for i, p in enumerate(res):
            w = np.asarray(p["w"], np.float32)
            bact.append(np.asarray(p["b"], np.float32) + np.einsum("okt,k->o", w, Cs[i]))
        m[f"h_lhsT{s}"] = np.ascontiguousarray(np.asarray(head["w"], np.float32).transpose(1, 2, 0)).astype(bf)
        m[f"r_lhsT{s}"] = np.ascontiguousarray(
            np.stack([np.asarray(p["w"], np.float32).transpose(1, 2, 0) for p in res], 1)
        ).astype(bf)  # (128, 7, 9, 128)
        Wfu = np.asarray(sp_["fusion"]["w"], np.float32)[:, :, 0]   # (256, 1024)
        bfu = np.asarray(sp_["fusion"]["b"], np.float32) + sum(
            Wfu[:, 128 * i:128 * (i + 1)] @ Cs[i] for i in range(8)
        )
        W0 = np.asarray(sp_["pred0"]["w"], np.float32)[:, :, 0]     # (256, 1280)
        W0g, W0s = W0[:, :256], W0[:, 256:]
        b0 = np.asarray(sp_["pred0"]["b"], np.float32) + sum(
            W0s[:, 128 * i:128 * (i + 1)] @ Cs[i] for i in range(8)
        )
        m[f"fu_lhsT{s}"] = np.ascontiguousarray(
            Wfu.T.reshape(8, 128, 2, 128).transpose(1, 0, 2, 3)
        ).astype(bf)
        m[f"p0_lhsT{s}"] = np.ascontiguousarray(
            W0s.T.reshape(8, 128, 2, 128).transpose(1, 0, 2, 3)
        ).astype(bf)
        m[f"g_lhsT{s}"] = np.ascontiguousarray(
            W0g.T.reshape(2, 128, 2, 128).transpose(1, 0, 2, 3)
        )
        m[f"p1_lhsT{s}"] = np.ascontiguousarray(
            np.asarray(sp_["pred1"]["w"], np.float32)[:, :, 0].T.reshape(2, 128, 64).transpose(1, 0, 2)
        ).astype(bf)
        m[f"p2_lhsT{s}"] = np.ascontiguousarray(
            np.asarray(sp_["pred2"]["w"], np.float32)[:, :, 0].T
        ).astype(bf)
        m[f"bact{s}"] = np.stack(bact, 1)          # (128, 8)
        m[f"binv{s}"] = np.stack(inv, 1)           # (128, 8)
        m[f"bfu{s}"] = np.ascontiguousarray(bfu.reshape(2, 128).T)
        m[f"b0{s}"] = np.ascontiguousarray(b0.reshape(2, 128).T)
        m[f"b1{s}"] = np.asarray(sp_["pred1"]["b"], np.float32).reshape(64, 1)
        m[f"b2{s}"] = np.asarray(sp_["pred2"]["b"], np.float32).reshape(2, 1)
    return m


def _prep_core(c, feat0, feat1, feat2, feat3, vertices, width, height):
    sl = slice(BPC * c, BPC * (c + 1))
    feats = [np.asarray(f[sl], np.float32) for f in (feat0, feat1, feat2, feat3)]
    tabs = _site_tables(feats)
    m = {
        "tab0a": tabs[0][0], "tab0b": tabs[0][1],
        "tab1": tabs[1].reshape(2 * SL[1], 512),
        "tab2": tabs[2].reshape(2 * SL[2], 1024),
        "tab3": tabs[3].reshape(2 * SL[3], 2048),
    }
    v = np.asarray(vertices[sl], np.float32)        # (2, 1024, 2)
    m["vertsrow"] = np.ascontiguousarray(v.transpose(2, 0, 1).reshape(2, NV))
    w = np.asarray(width[sl], np.float32)
    h = np.asarray(height[sl], np.float32)
    sc = np.zeros((4, 16, 2), np.float32)
    boff = np.zeros((4, 16), np.float32)
    for lev, (C, W) in enumerate(LEVELS):
        for jj in range(16):
            b_ = jj // 8
            sc[lev, jj, 0] = W / w[b_]
            sc[lev, jj, 1] = W / h[b_]
            if lev > 0:
                boff[lev, jj] = b_ * SL[lev]
    m["SC"] = np.broadcast_to(sc.reshape(1, 4, 32), (128, 4, 32)).copy()
    m["BOFF"] = np.broadcast_to(boff.reshape(1, 4, 16), (128, 4, 16)).copy()
    return m


_CACHE = {}


def _get_compiled():
    if "nc" not in _CACHE:
        nc = bacc.Bacc("TRN2", target_bir_lowering=False, debug=False, num_devices=NCORES)
        build_program(nc)
        nc.compile()
        _CACHE["nc"] = nc
    return _CACHE["nc"]


def make_in_maps(feat0, feat1, feat2, feat3, vertices, width, height, params):
    shared = _prep_shared(params)
    in_maps = []
    for c in range(NCORES):
        m = dict(shared)
        m.update(_prep_core(c, feat0, feat1, feat2, feat3, vertices, width, height))
        in_maps.append(m)
    return in_maps


def assemble_outputs(results):
    preds = []
    for s in range(3):
        full = np.empty((B, N, 2), np.float32)
        for c in range(NCORES):
            r = np.asarray(results[c][f"pred{s}"])      # (2, NV)
            full[BPC * c:BPC * (c + 1)] = r.reshape(2, BPC, N).transpose(1, 2, 0)
        preds.append(full)
    return tuple(preds)


def kernel(feat0, feat1, feat2, feat3, vertices, width, height, params):
    nc = _get_compiled()
    in_maps = make_in_maps(feat0, feat1, feat2, feat3, vertices, width, height, params)
    res = bass_utils.run_bass_kernel_spmd(nc, in_maps, core_ids=list(range(NCORES)))
    return assemble_outputs(res.results)
